# revision 1
# baseline (speedup 1.0000x reference)
"""Trainium2 Bass kernel for dual-input multi-head attention.

Computes, for each of two independent inputs x, y of shape [8, 1024, 768]:
    qkv = inp @ w_qkv.T ; split into 12 heads of 64
    attn = softmax(q k^T / sqrt(64)) v
    out  = attn @ w_proj.T + b_proj
Sharded data-parallel over the batch dim: core i handles batch i of x AND
batch i of y (16 batch-units over 8 cores = 2 per core).

Per-core design (measured ~338us vs the 385us predecessor):
  - Host pre-transposes and casts to bf16: inpT [C, N], w_qkvT [C, 3C]
    (columns REORDERED so the prologue's j-tiles 0/6 and the first v-chunk
    form one contiguous leading block), w_projT [C, C]. All matmuls run
    bf16 with fp32 PSUM accumulation.
  - ~150 dummy warm-up matmuls on a memset tile run while the input DMAs
    flow, so the PE's HAM clock gate is un-throttled (2.4 GHz) and the PE
    never idles long enough to re-throttle before real work starts. The
    startup-critical DMAs (leading wq block + x) are interleaved per
    chunk, with the first chunks split in half-columns to spread them
    over twice the DMA queues.
  - Attention runs in (head-pair, n-half) passes: scores are row-packed
    (contraction-64, two heads in PE row groups 0-63/64-127, adjacent
    emission -> concurrent, ~195ns/MM); exp on ScalarE straight out of
    PSUM ([128,1024] per call, 1/8 scale folded in; scores are O(+-15) so
    no max-subtraction is needed); P@V carries a 65th all-ones column so
    the softmax denominator falls out as PSUM row 64 for free. Score
    tiles double-buffer across key tiles so ScalarE exp stays one tile
    ahead of the P@V consumers.
    (Alternatives measured SLOWER on hardware: col-packed M=64 P@V pairs
    + 4-wide ones-matmul denominator quads lose to the ones-column trick
    because the extra denominator clusters cost real wall time while
    tile-packed concurrency only reaches ~2x; fp8 DoubleRow P@V halves
    the stream but costs 3e-2 relative error - over budget.)
  - Normalization is fused: the denominator row is staged to partition 0
    (custom-DVE ops misread PSUM at partition base 64), inverted with
    reciprocal_approx_fast (~5x cheaper than InstReciprocal), broadcast
    on the idle GpSimd engine, and applied by one tensor_tensor multiply
    that writes normalized bf16 attnT STRAIGHT from P@V PSUM - no
    separate PSUM-evacuation copy.
  - Cross-phase software pipelining: all QKV/projection matmul groups
    live in a tagged filler queue drained inside the attention passes
    (before each P@V group, where they hide the exp dependency wait).
    drain_until() forces prerequisite fillers (per key-tile v chunks,
    per-pair qk j-tiles) just in time, so attention of y starts while
    y's QKV is still streaming. proj(y) for token tiles 0-3 only needs
    the first n-half of attnT(y) and fills y's second-half passes; only
    token tiles 4-7 remain as the epilogue.
  - PSUM budget (8 banks): 2x score tiles (4) + 2x P@V (2) + filler (2).
    Prologue/epilogue matmul groups borrow the attention tags' banks
    (idle then) for deeper pipelining.
"""

from collections import deque

import numpy as np

import concourse.bacc as bacc
import concourse.mybir as mybir
import concourse.tile as tile
from concourse import bass_utils

B, N, C, H, HD = 8, 1024, 768, 12, 64
NT = N // 128  # 8 token tiles
CT = C // 128  # 6 contraction chunks
SCALE = HD ** -0.5
F32 = mybir.dt.float32
BF16 = mybir.dt.bfloat16
AF = mybir.ActivationFunctionType
ALU = mybir.AluOpType
N_CORES = 8

# Reordered w_qkvT column layout (host side builds this):
#   piece A (cols 0:1024):  jt0 | jt1 | jt6 | jt7 | v-cols 0:512
#   piece B (cols 1024:2304): jt2 | jt3 | jt8 | jt9 | jt4 | jt5 | jt10 | jt11 | v-cols 512:768
JT_OFF = {0: 0, 1: 128, 6: 256, 7: 384,
          2: 1024, 3: 1152, 8: 1280, 9: 1408,
          4: 1536, 5: 1664, 10: 1792, 11: 1920}
V_OFF = {0: 512, 1: 2048}  # v group g -> SBUF/DRAM column offset
TAG_BUFS = {"mm": 2, "pv": 2, "sc": 2}



def build_program():
    nc = bacc.Bacc("TRN2", target_bir_lowering=False, debug=False)
    inp_dram = [
        nc.dram_tensor("xT", [C, N], BF16, kind="ExternalInput"),
        nc.dram_tensor("yT", [C, N], BF16, kind="ExternalInput"),
    ]
    wqT = nc.dram_tensor("wqT", [C, 3 * C], BF16, kind="ExternalInput")
    wpT = nc.dram_tensor("wpT", [C, C], BF16, kind="ExternalInput")
    bp = nc.dram_tensor("bp", [1, C], F32, kind="ExternalInput")
    out_dram = [
        nc.dram_tensor("out_x", [N, C], F32, kind="ExternalOutput"),
        nc.dram_tensor("out_y", [N, C], F32, kind="ExternalOutput"),
    ]

    with tile.TileContext(nc) as tc:
        with (
            tc.tile_pool(name="pers", bufs=1) as pers,
            tc.tile_pool(name="dbl", bufs=2) as dbl,
            tc.tile_pool(name="pexp", bufs=4) as pep,
            tc.tile_pool(name="small", bufs=2) as smp,
            tc.tile_pool(name="rbsb", bufs=4) as rbsbp,
            tc.tile_pool(name="outp", bufs=2) as outp,
            tc.tile_pool(name="ps", bufs=1, space="PSUM") as ps,
        ):
            # PE warm-up: dummy matmuls on a memset tile while input DMAs run,
            # so HAM un-throttles (1.2 -> 2.4 GHz) before real work arrives.
            wu = pers.tile([128, 128], BF16, name="wu")
            nc.vector.memset(wu[:], 0.125)
            wu_ps = ps.tile([128, 128], F32, name="wu_ps", tag="mm", bufs=2)
            for _ in range(150):
                nc.tensor.matmul(wu_ps[:], wu[:], wu[:], start=True, stop=True)

            # startup-critical DMAs first: the leading wq block (prologue
            # j-tiles 0/1/6/7 + v group 0) interleaved with x chunks.
            wq_sb = pers.tile([128, CT, 3 * C], BF16, name="wq_sb")
            inp_sb = {
                0: dbl.tile([128, CT, N], BF16, name="inp_sb", tag="inp"),
                1: dbl.tile([128, CT, N], BF16, name="inp_sb2", tag="inp"),
            }
            for c in range(CT):
                if c < 2:
                    # first chunks split in half-columns: twice the DMA queues
                    # in parallel on the startup-critical transfers
                    for h in range(2):
                        nc.sync.dma_start(
                            wq_sb[:, c, h * 512 : (h + 1) * 512],
                            wqT[c * 128 : (c + 1) * 128, h * 512 : (h + 1) * 512],
                        )
                        nc.sync.dma_start(
                            inp_sb[0][:, c, h * 512 : (h + 1) * 512],
                            inp_dram[0][c * 128 : (c + 1) * 128, h * 512 : (h + 1) * 512],
                        )
                else:
                    nc.sync.dma_start(
                        wq_sb[:, c, 0:1024], wqT[c * 128 : (c + 1) * 128, 0:1024]
                    )
                    nc.sync.dma_start(
                        inp_sb[0][:, c, :], inp_dram[0][c * 128 : (c + 1) * 128, :]
                    )
            for c in range(CT):
                nc.sync.dma_start(
                    wq_sb[:, c, 1024 : 3 * C],
                    wqT[c * 128 : (c + 1) * 128, 1024 : 3 * C],
                )
            for c in range(CT):
                nc.sync.dma_start(
                    inp_sb[1][:, c, :], inp_dram[1][c * 128 : (c + 1) * 128, :]
                )
            wp_sb = pers.tile([128, CT, C], BF16, name="wp_sb")
            for c in range(CT):
                nc.sync.dma_start(wp_sb[:, c, :], wpT[c * 128 : (c + 1) * 128, :])
            b_row = pers.tile([1, C], F32, name="b_row")
            nc.sync.dma_start(b_row[:], bp[:, :])
            bias_sb = pers.tile([128, C], F32, name="bias_sb")
            nc.gpsimd.partition_broadcast(bias_sb[:], b_row[:1, :])

            qkT_sb, v_sb, attnT_sb = {}, {}, {}
            for idx in range(2):
                # q,k transposed: j-tiles 0..5 = q (2 heads/tile), 6..11 = k
                qkT_sb[idx] = dbl.tile([128, H, N], BF16, name="qkT_sb", tag="qkT")
                # v per (token-tile, head): 64 cols of v then one col of ones
                v_sb[idx] = dbl.tile([128, NT, H, HD + 1], BF16, name="v_sb", tag="v")
                nc.vector.memset(v_sb[idx][:, :, :, HD : HD + 1], 1.0)
                # attention output, transposed [C, N] as 6 chunks of 128
                attnT_sb[idx] = dbl.tile([128, CT, N], BF16, name="attnT_sb", tag="attnT")

            # prologue/epilogue matmul groups rotate through the attention
            # tags' PSUM banks (idle then); attention-phase fillers use "mm".
            PSUM_CYCLE = ("mm", "pv", "mm", "pv")
            _tag_n = [0]

            def next_tag(borrow):
                if not borrow:
                    return "mm"
                t = PSUM_CYCLE[_tag_n[0] % len(PSUM_CYCLE)]
                _tag_n[0] += 1
                return t

            def emit_qkT(idx, jt, copy_engine, borrow=False):
                for _ in gen_qkT(idx, jt, copy_engine, borrow):
                    pass

            def gen_qkT(idx, jt, copy_engine, borrow=False):
                # qkvT[j, n] = sum_c w_qkvT[c, j] inpT[c, n]
                off = JT_OFF[jt]
                for g in range(2):
                    tg = next_tag(borrow)
                    ps_qk = ps.tile([128, 512], F32, name="ps_qk", tag=tg, bufs=TAG_BUFS[tg])
                    for c in range(CT):
                        nc.tensor.matmul(
                            ps_qk[:],
                            wq_sb[:, c, off : off + 128],
                            inp_sb[idx][:, c, g * 512 : (g + 1) * 512],
                            start=(c == 0),
                            stop=(c == CT - 1),
                        )
                        yield
                    dst = qkT_sb[idx][:, jt, g * 512 : (g + 1) * 512]
                    if copy_engine == "act":
                        nc.scalar.copy(dst, ps_qk[:])
                    else:
                        nc.vector.tensor_copy(dst, ps_qk[:])

            def emit_v(idx, nt, g, copy_engine, borrow=False):
                for _ in gen_v(idx, nt, g, copy_engine, borrow):
                    pass

            def gen_v(idx, nt, g, copy_engine, borrow=False):
                # v[n, j] = sum_c inpT[c, n] w_qkvT[c, 2C + j]
                w = 512 if g == 0 else 256
                off = V_OFF[g]
                tg = next_tag(borrow)
                ps_v = ps.tile([128, 512], F32, name="ps_v", tag=tg, bufs=TAG_BUFS[tg])
                for c in range(CT):
                    nc.tensor.matmul(
                        ps_v[:, :w],
                        inp_sb[idx][:, c, nt * 128 : (nt + 1) * 128],
                        wq_sb[:, c, off : off + w],
                        start=(c == 0),
                        stop=(c == CT - 1),
                    )
                    yield
                hview = ps_v[:, :w].rearrange("p (h d) -> p h d", d=HD)
                dst = v_sb[idx][:, nt, g * 8 : g * 8 + w // HD, 0:HD]
                if copy_engine == "act":
                    nc.scalar.copy(dst, hview)
                else:
                    nc.vector.tensor_copy(dst, hview)

            def emit_proj(idx, nt, borrow=False):
                for _ in gen_proj(idx, nt, borrow):
                    pass

            def gen_proj(idx, nt, borrow=False):
                # p1's bias-add is emitted BEFORE p2 is allocated: with the
                # 1-buffer "mm" tag, p2's slot reuse needs p1's reader already
                # in the schedule or the WAR dependency deadlocks.
                out_sb = outp.tile([128, C], F32, name="out_sb", tag="outsb")
                p1 = ps.tile([128, 512], F32, name="p1", tag=(t1 := next_tag(borrow)), bufs=TAG_BUFS[t1])
                for c in range(CT):
                    nc.tensor.matmul(
                        p1[:],
                        attnT_sb[idx][:, c, nt * 128 : (nt + 1) * 128],
                        wp_sb[:, c, 0:512],
                        start=(c == 0),
                        stop=(c == CT - 1),
                    )
                    yield
                nc.vector.tensor_tensor(
                    out_sb[:, 0:512], p1[:], bias_sb[:, 0:512], op=ALU.add
                )
                p2 = ps.tile([128, 512], F32, name="p2", tag=(t2 := next_tag(borrow)), bufs=TAG_BUFS[t2])
                for c in range(CT):
                    nc.tensor.matmul(
                        p2[:, :256],
                        attnT_sb[idx][:, c, nt * 128 : (nt + 1) * 128],
                        wp_sb[:, c, 512:768],
                        start=(c == 0),
                        stop=(c == CT - 1),
                    )
                    yield
                nc.vector.tensor_tensor(
                    out_sb[:, 512:768], p2[:, :256], bias_sb[:, 512:768], op=ALU.add
                )
                nc.sync.dma_start(out_dram[idx][nt * 128 : (nt + 1) * 128, :], out_sb[:])

            fillers = deque()  # (generator, tag) yielding once per PE matmul
            done_tags = set()

            def drain_mm(k):
                # advance filler work by k PE matmuls
                while k > 0 and fillers:
                    try:
                        next(fillers[0][0])
                        k -= 1
                    except StopIteration:
                        done_tags.add(fillers[0][1])
                        fillers.popleft()

            def drain_until(tag):
                # emit filler work until the generator tagged `tag` finishes
                while fillers and tag not in done_tags:
                    try:
                        next(fillers[0][0])
                    except StopIteration:
                        done_tags.add(fillers[0][1])
                        fillers.popleft()

            def drain_all():
                while fillers:
                    try:
                        next(fillers[0][0])
                    except StopIteration:
                        done_tags.add(fillers[0][1])
                        fillers.popleft()

            def emit_norm(idx, t, g, pvs):
                # per head: fast-reciprocal of the ones-column denominator row
                # (row HD of the P@V PSUM), GpSimd-broadcast to [64,512], and
                # multiply P@V PSUM straight into the bf16 transposed
                # attention buffer (fused copy+normalize).
                gs = slice(g * 512, (g + 1) * 512)
                for (_, ab), pv in pvs.items():
                    pb = ab * 64
                    # custom-DVE recip mishandles PSUM reads at partition
                    # base 64 on HW: stage the denominator row to partition 0
                    # in SBUF first with a plain copy.
                    dn = smp.tile([1, 512], F32, name="dn", tag=f"dn{ab}")
                    nc.vector.tensor_copy(dn[0:1, :], pv[HD : HD + 1, :])
                    rc = smp.tile([1, 512], F32, name="rc", tag=f"rc{ab}")
                    nc.vector.reciprocal_approx_fast(rc[0:1, :], dn[0:1, :])
                    rb = rbsbp.tile([64, 512], F32, name="rb", tag="rb")
                    nc.gpsimd.partition_broadcast(rb[:], rc[0:1, :])
                    nc.vector.tensor_tensor(
                        attnT_sb[idx][pb : pb + 64, t, gs],
                        pv[0:HD, :],
                        rb[:],
                        op=ALU.mult,
                    )

            def emit_attn_pair(idx, t, g):
                # 2 heads per pass: scores row-packed (two heads in PE row
                # groups 0-63/64-127, adjacent emission); P@V with the
                # ones-column (M=65) emitting the softmax denominator as the
                # 65th row for free. Score tiles double-buffer across mt so
                # ScalarE exp stays ahead of the P@V consumers.
                pvs = {}
                for ab in range(2):
                    pvs[(t, ab)] = ps.tile(
                        [HD + 1, 512], F32, name="pv", tag="pv", bufs=2
                    )

                def sc_exp(t, mt):
                    sc = ps.tile([128, 2, 512], F32, name="sc", tag="sc", bufs=2)
                    for ab in range(2):
                        pb = ab * 64
                        nc.tensor.matmul(
                            sc[:, ab, :],
                            qkT_sb[idx][pb : pb + 64, 6 + t, mt * 128 : (mt + 1) * 128],
                            qkT_sb[idx][pb : pb + 64, t, g * 512 : (g + 1) * 512],
                            start=True,
                            stop=True,
                            tile_position=(pb, 0),
                        )
                    pe = pep.tile([128, N], BF16, name="pe", tag="pexp")
                    nc.scalar.activation(
                        pe[:],
                        sc[:].rearrange("p a b -> p (a b)"),
                        AF.Exp,
                        scale=SCALE,
                    )
                    return pe

                def emit_pv(mt, pe):
                    for ab in range(2):
                        nc.tensor.matmul(
                            pvs[(t, ab)][:, :],
                            v_sb[idx][:, mt, 2 * t + ab, :],
                            pe[:, ab * 512 : (ab + 1) * 512],
                            start=(mt == 0),
                            stop=(mt == NT - 1),
                        )

                # P@V lags the score/exp pipeline by one key tile: slot mt
                # emits pv(mt-1), whose exp finished a whole slot earlier, so
                # the PE never waits on ScalarE at the P@V handoff. Fillers
                # drain BEFORE each P@V group (after it they would sit behind
                # it in the in-order PE queue).
                pe_prev = None
                pe_cur = sc_exp(t, 0)
                for mt in range(NT):
                    pe_next = sc_exp(t, mt + 1) if mt + 1 < NT else None
                    drain_until(f"v{idx}_{0 if t < 4 else 1}_{mt}")
                    drain_mm(2)
                    if mt > 0:
                        emit_pv(mt - 1, pe_prev)
                    pe_prev, pe_cur = pe_cur, pe_next
                drain_mm(4)
                emit_pv(NT - 1, pe_prev)
                emit_norm(idx, t, g, pvs)

            # ---- pipelined emission ----
            # prologue: only what attn(x) quad 0 needs — q/k j-tiles 0,1,6,7
            # plus v group 0 (heads 0-7); the rest of QKV(x) drains as filler
            # inside the attention passes
            for jt in (0, 6):
                emit_qkT(0, jt, "act", borrow=True)
            for nt in range(NT):
                emit_v(0, nt, 0, "act", borrow=True)
            done_tags.update({"qk0_0", "qk0_6"})  # prologue-emitted, not fillers
            # fillers, in first-use order: x pairs 1-5 j-tiles, x v-tail,
            # then y's prologue-equivalent, y's tail
            for jt in (1, 7, 2, 8, 3, 9):
                fillers.append((gen_qkT(0, jt, "dve"), f"qk0_{jt}"))
            for nt in range(NT):
                fillers.append((gen_v(0, nt, 1, "dve"), f"v0_1_{nt}"))
            for jt in (4, 10, 5, 11):
                fillers.append((gen_qkT(0, jt, "dve"), f"qk0_{jt}"))
            for jt in (0, 6, 1, 7):
                fillers.append((gen_qkT(1, jt, "dve"), f"qk1_{jt}"))
            for nt in range(NT):
                fillers.append((gen_v(1, nt, 0, "dve"), f"v1_0_{nt}"))
            for jt in (2, 8, 3, 9, 4, 10, 5, 11):
                fillers.append((gen_qkT(1, jt, "dve"), f"qk1_{jt}"))

            def attn_input(idx):
                for g in range(2):
                    for t in range(H // 2):
                        drain_until(f"qk{idx}_{t}")
                        drain_until(f"qk{idx}_{6 + t}")
                        emit_attn_pair(idx, t, g)
                    if idx == 1 and g == 0:
                        # proj(y) for the first 4 token tiles only needs the
                        # g=0 half of attnT: fill y's g=1 passes with it
                        for nt in range(4):
                            fillers.append((gen_proj(1, nt), f"pj1_{nt}"))

            done_tags.update(f"v0_0_{nt}" for nt in range(NT))
            attn_input(0)
            # attn(y) with v(y) tail chunks + proj(x) drained in
            for nt in range(NT):
                fillers.append((gen_v(1, nt, 1, "dve"), f"v1_1_{nt}"))
            for nt in range(NT):
                fillers.append((gen_proj(0, nt), f"pj0_{nt}"))
            attn_input(1)
            drain_all()
            for nt in range(4, NT):
                emit_proj(1, nt, borrow=True)

    nc.compile()
    return nc


_PROGRAM = None


def _get_program():
    global _PROGRAM
    if _PROGRAM is None:
        _PROGRAM = build_program()
    return _PROGRAM


def _reorder_wq_cols(wqT):
    # wqT is [C, 3C] (w_qkv.T). Build the column order described by
    # JT_OFF/V_OFF: [jt0 jt1 jt6 jt7 | v 0:512 | jt2 jt3 jt8 jt9 jt4 jt5
    # jt10 jt11 | v 512:768].
    cols = []
    for jt in (0, 1, 6, 7):
        cols.append(wqT[:, jt * 128 : (jt + 1) * 128])
    cols.append(wqT[:, 2 * C : 2 * C + 512])
    for jt in (2, 3, 8, 9, 4, 5, 10, 11):
        cols.append(wqT[:, jt * 128 : (jt + 1) * 128])
    cols.append(wqT[:, 2 * C + 512 : 3 * C])
    return np.concatenate(cols, axis=1)


def make_in_maps(x, y, w_qkv, w_proj, b_proj):
    import ml_dtypes

    bf = ml_dtypes.bfloat16
    x = np.asarray(x, np.float32)
    y = np.asarray(y, np.float32)
    xT = np.ascontiguousarray(x.transpose(0, 2, 1)).astype(bf)
    yT = np.ascontiguousarray(y.transpose(0, 2, 1)).astype(bf)
    wqT = np.ascontiguousarray(
        _reorder_wq_cols(np.asarray(w_qkv, np.float32).T)
    ).astype(bf)
    wpT = np.ascontiguousarray(np.asarray(w_proj, np.float32).T).astype(bf)
    bpv = np.ascontiguousarray(np.asarray(b_proj, np.float32).reshape(1, C))
    return [
        {"xT": xT[i], "yT": yT[i], "wqT": wqT, "wpT": wpT, "bp": bpv}
        for i in range(N_CORES)
    ]


def kernel(x, y, w_qkv, w_proj, b_proj):
    nc = _get_program()
    in_maps = make_in_maps(x, y, w_qkv, w_proj, b_proj)
    res = bass_utils.run_bass_kernel_spmd(nc, in_maps, core_ids=list(range(N_CORES)))
    xo = np.stack([np.asarray(res.results[i]["out_x"]) for i in range(N_CORES)])
    yo = np.stack([np.asarray(res.results[i]["out_y"]) for i in range(N_CORES)])
    return (xo, yo)

